# revision 17
# baseline (speedup 1.0000x reference)
"""BlendShapes model kernel for 8 Trainium2 NeuronCores.

Computation (reference):
    pose_repr = pose[:, 1:].reshape(B, 23, 9) - eye      # (B, J, 9)
    per-joint MLP 9 -> 18 -> 32 -> 8 (ReLU between)      # coff (B, J, 8)
    basis_full = basis[:, None] * mask[:, :, None, None]  # (V, J, 8, 3)
    res = einsum('bjk,vjkc->bvc', coff, basis_full)       # (B, V, 3)

Mapping:
  - Vertices are sharded across the 8 cores; each core owns 512 "A" slots
    and 352 "B" slots (864 * 8 = 6912 >= V). A-slots hold vertices whose
    mask row is all ones: for those res = einsum('bk,vkc->bvc', S, basis)
    with S[b,k] = sum_j coff[b,j,k], a K=8 matmul that streams each output
    column ONCE instead of twice (K=184 split 128+56). All other vertices
    (and all-ones overflow) go to B-slots computed with the exact
    mask-folded K=184 path. The host assembles per-core basisA / bfmB
    panels (scaled by 2^13 so f16 stays normal; basis ~1e-4) and scatters
    the result back by vertex index.
  - S is computed on the PE with a ones-selection matrix packed next to the
    MLP weights (K=128 + K=56 accumulated).
  - The eye subtraction is folded into the L1 bias on the host.
  - MLP joints are processed in chunks of 4 (3 for the tail) with
    block-diagonal weights; pose/h1/h2 live in wide tiles with one col
    block per chunk. Odd L1 chunks sit at partition base 64 (pose packed
    [100, 3072]) so the pose DMA spans ~13 of 16 SDMA engines instead
    of 5. L3 outputs land at 32-row offsets of two shared PSUM tiles
    (matmul tile_position), so one bias-add epilogue per half yields
    coffT directly.
  - PSUM-accumulated K chunks evacuate via ACT/DVE with the exact 2^-13
    descale into f16 output (upcast to f32 on host). Input and output DMAs
    are split across the two HWDGE queues (sync + scalar).
"""

import numpy as np

N_VERT, N_JOINT, BPJ, BATCH = 6890, 23, 8, 1024
VC = 864  # vertex slots per core: 512 A + 352 B
VC3 = VC * 3  # 2592
NA, NB_SLOTS = 512, 352
A3, B3 = NA * 3, NB_SLOTS * 3  # 1536, 1056
NB = BATCH // 128  # 8 b-tiles

# Unified joint chunking for the MLP block-diagonal weights.
CHUNKS = [(0, 4), (4, 8), (8, 12), (12, 16), (16, 20), (20, 23)]


def _offsets(mpj):
    offs, col = [], 0
    for js, je in CHUNKS:
        offs.append(col)
        col += (je - js) * mpj
    return offs, col


W1_OFF, W1_TOT = _offsets(18)  # 414
W2_OFF, W2_TOT = _offsets(32)  # 736
W3_OFF, W3_TOT = _offsets(8)   # 184
W2_OFF = [W1_TOT + o for o in W2_OFF]
W3_OFF = [W1_TOT + W2_TOT + o for o in W3_OFF]
EA_OFF = W1_TOT + W2_TOT + W3_TOT  # ones matrix [128, 8] for S (K=128 part)
EB_OFF = EA_OFF + 8                # ones matrix [56, 8] (K=56 part)
W_COLS = EB_OFF + 8  # 1350

# bias_all columns: [0:6] L1 bias (eye folded in), [6:12] L2 bias,
# [12] L3 bias chunks 0-3 stacked at 32-row strides, [13] chunks 4-5.
BIAS_COLS = 14
BSCALE = 8192.0  # 2**13, exact in f16
DESCALE = 1.0 / 8192.0  # exact in f32

_CACHED = {}


def _build_nc():
    import concourse.tile as tile
    from concourse import bacc, mybir
    from contextlib import ExitStack

    dt = mybir.dt
    f32, f16 = dt.float32, dt.float16
    AF = mybir.ActivationFunctionType
    ALU = mybir.AluOpType

    nc = bacc.Bacc(None, target_bir_lowering=False)

    pose_t = nc.dram_tensor("pose_t", [100, 3072], f16, kind="ExternalInput")
    basisa_t = nc.dram_tensor("basisa_t", [BPJ, A3], f16, kind="ExternalInput")
    bfmb_a_t = nc.dram_tensor("bfmb_a_t", [128, B3], f16, kind="ExternalInput")
    bfmb_b_t = nc.dram_tensor("bfmb_b_t", [56, B3], f16, kind="ExternalInput")
    w_all_t = nc.dram_tensor("w_all", [128, W_COLS], f16, kind="ExternalInput")
    bias_t = nc.dram_tensor("bias_all", [128, BIAS_COLS], f32, kind="ExternalInput")
    res = nc.dram_tensor("res", [BATCH, VC3], f16, kind="ExternalOutput")

    with ExitStack() as ctx:
        tc = ctx.enter_context(tile.TileContext(nc))
        const = ctx.enter_context(tc.tile_pool(name="const", bufs=1))
        work = ctx.enter_context(tc.tile_pool(name="work", bufs=1))
        outp = ctx.enter_context(tc.tile_pool(name="outp", bufs=3))
        pmlp = ctx.enter_context(tc.tile_pool(name="pmlp", bufs=4, space="PSUM"))
        pmain = ctx.enter_context(tc.tile_pool(name="pmain", bufs=2, space="PSUM"))

        # ---- input DMAs split across the two HWDGE queues, critical-path
        # tensors first on each. pose rows [0:36] + [64:100] land on
        # disjoint SDMA engine groups; the weights stream layer by layer so
        # L1 can start as soon as its own block arrives.
        pose_sb = const.tile([100, 3072], f16, tag="pose")
        nc.sync.dma_start(out=pose_sb[0:36, :], in_=pose_t[0:36, :])
        nc.sync.dma_start(out=pose_sb[64:100, :], in_=pose_t[64:100, :])
        w_sb = const.tile([128, W_COLS], f16, tag="w")
        nc.scalar.dma_start(out=w_sb[:, 0:W1_TOT], in_=w_all_t[:, 0:W1_TOT])
        bias_sb = const.tile([128, BIAS_COLS], f32, tag="bias")
        nc.scalar.dma_start(out=bias_sb[:], in_=bias_t[:, :])
        nc.scalar.dma_start(
            out=w_sb[:, W1_TOT:W_COLS], in_=w_all_t[:, W1_TOT:W_COLS]
        )
        bfmb_a = const.tile([128, B3], f16, tag="bfmb_a")
        nc.scalar.dma_start(out=bfmb_a[:], in_=bfmb_a_t[:, :])
        bfmb_b = const.tile([56, B3], f16, tag="bfmb_b")
        nc.scalar.dma_start(out=bfmb_b[:], in_=bfmb_b_t[:, :])
        basisa = const.tile([BPJ, A3], f16, tag="basisa")
        nc.scalar.dma_start(out=basisa[:], in_=basisa_t[:, :])

        h1 = work.tile([72, 6 * 1024], f16, tag="h1")
        h2 = work.tile([128, 6 * 1024], f16, tag="h2")
        coffT_a = work.tile([128, BATCH], f16, tag="coffT_a")
        coffT_b = work.tile([56, BATCH], f16, tag="coffT_b")
        st_sb = work.tile([BPJ, BATCH], f16, tag="st")

        def mlp_epilogue(use_act, dst, ps, bias_ap):
            # ReLU(x + b); split between ACT and DVE so the PSUM chain
            # advances two tiles per epilogue latency.
            if use_act:
                nc.scalar.activation(dst, ps, AF.Relu, bias=bias_ap)
            else:
                nc.vector.tensor_scalar(
                    out=dst, in0=ps, scalar1=bias_ap, scalar2=0.0,
                    op0=ALU.add, op1=ALU.max,
                )

        # even chunks first: they only need the first pose DMA slice, so L1
        # starts ~1.3 us before the odd slice lands.
        CORDER = [0, 2, 4, 1, 3, 5]

        def mlp_half(h):
            # L1: 9nj -> 18nj. Odd chunks live at partition base 64 in both
            # pose and w so the pose DMA spans more partitions.
            for ci, c in enumerate(CORDER):
                js, je = CHUNKS[c]
                nj = je - js
                K, M = 9 * nj, 18 * nj
                off = W1_OFF[c]
                r0 = 64 * (c % 2)
                cs = slice(1024 * (c // 2) + 512 * h, 1024 * (c // 2) + 512 * h + 512)
                hcs = slice(1024 * c + 512 * h, 1024 * c + 512 * h + 512)
                ps = pmlp.tile([128, 512], f32, tag="psmlp", name=f"ps1_{c}_{h}")
                nc.tensor.matmul(
                    ps[0:M, :], lhsT=w_sb[r0 : r0 + K, off : off + M],
                    rhs=pose_sb[r0 : r0 + K, cs], start=True, stop=True,
                    tile_position=(r0, 0),
                )
                mlp_epilogue(ci % 2 == 0, h1[0:M, hcs], ps[0:M, :], bias_sb[0:M, c : c + 1])
            # L2: 18nj -> 32nj
            for ci, c in enumerate(CORDER):
                js, je = CHUNKS[c]
                nj = je - js
                K, M = 18 * nj, 32 * nj
                off = W2_OFF[c]
                hcs = slice(1024 * c + 512 * h, 1024 * c + 512 * h + 512)
                ps = pmlp.tile([128, 512], f32, tag="psmlp", name=f"ps2_{c}_{h}")
                nc.tensor.matmul(
                    ps[0:M, :], lhsT=w_sb[0:K, off : off + M], rhs=h1[0:K, hcs],
                    start=True, stop=True,
                )
                mlp_epilogue(ci % 2 == 1, h2[0:M, hcs], ps[0:M, :], bias_sb[0:M, 6 + c : 7 + c])
            # L3: 32nj -> 8nj, written at 32-row offsets of two shared PSUM
            # tiles so the (j, k) rows of coffT form directly; one bias-add
            # epilogue per tile replaces per-chunk epilogues + merge DMAs.
            psA = pmlp.tile([128, 512], f32, tag="psmlp", name=f"ps3a_{h}")
            psB = pmlp.tile([128, 512], f32, tag="psmlp", name=f"ps3b_{h}")
            for c in CORDER:
                js, je = CHUNKS[c]
                nj = je - js
                K, M = 32 * nj, 8 * nj
                off = W3_OFF[c]
                hcs = slice(1024 * c + 512 * h, 1024 * c + 512 * h + 512)
                dstp, r0 = (psA, 32 * c) if c < 4 else (psB, 32 * (c - 4))
                nc.tensor.matmul(
                    dstp[r0 : r0 + M, :], lhsT=w_sb[0:K, off : off + M],
                    rhs=h2[0:K, hcs], start=True, stop=True,
                    tile_position=(0, r0),
                )
            hs = slice(512 * h, 512 * h + 512)
            nc.vector.tensor_scalar(
                out=coffT_a[:, hs], in0=psA[:], scalar1=bias_sb[0:128, 12:13],
                scalar2=None, op0=ALU.add,
            )
            nc.vector.tensor_scalar(
                out=coffT_b[:, hs], in0=psB[0:56, :], scalar1=bias_sb[0:56, 13:14],
                scalar2=None, op0=ALU.add,
            )
            # S[b,k] = sum_j coff[b,j,k] via ones matmuls (for the A path).
            ps_st = pmlp.tile([128, 512], f32, tag="psmlp", name=f"ps_st_{h}")
            nc.tensor.matmul(
                ps_st[0:BPJ, :], lhsT=w_sb[0:128, EA_OFF : EA_OFF + BPJ],
                rhs=coffT_a[:, hs], start=True, stop=False,
            )
            nc.tensor.matmul(
                ps_st[0:BPJ, :], lhsT=w_sb[0:56, EB_OFF : EB_OFF + BPJ],
                rhs=coffT_b[:, hs], start=False, stop=True,
            )
            nc.vector.tensor_scalar(
                out=st_sb[:, hs], in0=ps_st[0:BPJ, :], scalar1=1.0,
                scalar2=None, op0=ALU.mult,
            )

        def main_btile(bt):
            bsl = slice(bt * 128, (bt + 1) * 128)
            ostrip = outp.tile([128, VC3], f16, tag="ostrip", name=f"ostrip_{bt}")

            def evac(k, osl, ps, psl):
                if k % 2 == 0:
                    nc.scalar.activation(ostrip[:, osl], ps[:, psl], AF.Copy, scale=DESCALE)
                else:
                    nc.vector.tensor_scalar(
                        out=ostrip[:, osl], in0=ps[:, psl], scalar1=DESCALE,
                        scalar2=None, op0=ALU.mult,
                    )

            k0 = bt * 3
            # B-path pairs first (need only coffT), the ST-dependent A-path
            # last, so the S evacuation latency hides behind B matmuls.
            # pair 2: B columns 512:1056 (512 + 32 tail), two K passes with
            # shared weight loads per pass
            ps = pmain.tile([128, 1024], f32, tag="ps", name=f"ps_{bt}_2")
            nc.tensor.matmul(
                ps[:, 0:512], lhsT=coffT_a[:, bsl], rhs=bfmb_a[:, 512:1024],
                start=True, stop=False,
            )
            nc.tensor.matmul(
                ps[:, 512:544], lhsT=coffT_a[:, bsl], rhs=bfmb_a[:, 1024:1056],
                start=True, stop=False,
            )
            nc.tensor.matmul(
                ps[:, 0:512], lhsT=coffT_b[:, bsl], rhs=bfmb_b[:, 512:1024],
                start=False, stop=True,
            )
            nc.tensor.matmul(
                ps[:, 512:544], lhsT=coffT_b[:, bsl], rhs=bfmb_b[:, 1024:1056],
                start=False, stop=True,
            )
            evac(k0 + 2, slice(2048, 2592), ps, slice(0, 544))
            # pair 1: B columns 0:512 (bank 1) + A columns 1024:1536 (bank 0)
            ps = pmain.tile([128, 1024], f32, tag="ps", name=f"ps_{bt}_1")
            nc.tensor.matmul(
                ps[:, 512:1024], lhsT=coffT_a[:, bsl], rhs=bfmb_a[:, 0:512],
                start=True, stop=False,
            )
            nc.tensor.matmul(
                ps[:, 512:1024], lhsT=coffT_b[:, bsl], rhs=bfmb_b[:, 0:512],
                start=False, stop=True,
            )
            nc.tensor.matmul(
                ps[:, 0:512], lhsT=st_sb[:, bsl], rhs=basisa[:, 1024:1536],
                start=True, stop=True,
            )
            evac(k0 + 1, slice(1024, 2048), ps, slice(0, 1024))
            # pair 0: A columns 0:1024, K=8 single pass
            ps = pmain.tile([128, 1024], f32, tag="ps", name=f"ps_{bt}_0")
            nc.tensor.matmul(
                ps[:, 0:512], lhsT=st_sb[:, bsl], rhs=basisa[:, 0:512],
                start=True, stop=True,
            )
            nc.tensor.matmul(
                ps[:, 512:1024], lhsT=st_sb[:, bsl], rhs=basisa[:, 512:1024],
                start=True, stop=True,
            )
            evac(k0, slice(0, 1024), ps, slice(0, 1024))

            # output DMA: one per b-tile, alternating queues; the last two
            # b-tiles stream per-pair strips to shorten the tail.
            if bt < 6:
                eng = nc.sync if bt % 2 == 0 else nc.scalar
                eng.dma_start(out=res[bsl, :], in_=ostrip[:])
            else:
                for p, (o0, o1) in enumerate([(0, 1024), (1024, 2048), (2048, 2592)]):
                    eng = nc.sync if (bt + p) % 2 == 0 else nc.scalar
                    eng.dma_start(out=res[bsl, o0:o1], in_=ostrip[:, o0:o1])

        mlp_half(0)
        for bt in range(4):
            main_btile(bt)
        mlp_half(1)
        for bt in range(4, NB):
            main_btile(bt)

    nc.finalize()
    return nc


def _pack_host(pose, basis, mask, w1, b1, w2, b2, w3, b3):
    B = BATCH
    pr = pose[:, 1:].reshape(B, N_JOINT, 9)  # (b, j, i)
    pose_t = np.zeros((100, 3072), np.float16)
    for c, (js, je) in enumerate(CHUNKS):
        nj = je - js
        r0 = 64 * (c % 2)
        c0 = 1024 * (c // 2)
        pose_t[r0 : r0 + 9 * nj, c0 : c0 + 1024] = (
            pr[:, js:je, :].reshape(B, 9 * nj).T.astype(np.float16)
        )
    # rows 36:64 unused (partition alignment for the odd-chunk stack)

    # vertex assignment: all-ones-mask vertices -> A slots (K=8 path),
    # everything else (+ overflow) -> B slots (exact mask-folded path).
    ones_v = np.nonzero((mask > 0.5).all(axis=1))[0]
    rest_v = np.nonzero(~(mask > 0.5).all(axis=1))[0]
    a_sel = ones_v[: 8 * NA]
    b_sel = np.concatenate([ones_v[8 * NA :], rest_v])
    if len(b_sel) > 8 * NB_SLOTS:
        raise ValueError(
            f"B-slot overflow: {len(b_sel)} masked vertices > {8 * NB_SLOTS} slots"
        )
    # round-robin-balanced contiguous split across cores
    a_cnt = [len(a_sel) // 8 + (1 if i < len(a_sel) % 8 else 0) for i in range(8)]
    b_cnt = [len(b_sel) // 8 + (1 if i < len(b_sel) % 8 else 0) for i in range(8)]
    assert max(b_cnt) <= NB_SLOTS and max(a_cnt) <= NA
    a_lists, b_lists = [], []
    ai = bi = 0
    for i in range(8):
        a_lists.append(a_sel[ai : ai + a_cnt[i]])
        b_lists.append(b_sel[bi : bi + b_cnt[i]])
        ai += a_cnt[i]
        bi += b_cnt[i]

    basisT3 = basis.transpose(1, 0, 2).reshape(BPJ, N_VERT * 3)  # (8, V*3) rows k
    m3 = np.repeat(mask.T, 3, axis=1)  # (23, V*3)
    bfm_full = (m3[:, None, :] * basisT3[None, :, :] * BSCALE).reshape(
        N_JOINT * BPJ, N_VERT * 3
    )
    basis_s = basisT3 * BSCALE

    basisa_l, bfma_l, bfmb_l = [], [], []
    for i in range(8):
        av, bv = a_lists[i], b_lists[i]
        acols = (av[:, None] * 3 + np.arange(3)[None, :]).reshape(-1)
        bcols = (bv[:, None] * 3 + np.arange(3)[None, :]).reshape(-1)
        ba = np.zeros((BPJ, A3), np.float16)
        ba[:, : len(acols)] = basis_s[:, acols]
        bb = np.zeros((N_JOINT * BPJ, B3), np.float16)
        bb[:, : len(bcols)] = bfm_full[:, bcols]
        basisa_l.append(ba)
        bfma_l.append(np.ascontiguousarray(bb[0:128]))
        bfmb_l.append(np.ascontiguousarray(bb[128:184]))

    w_all = np.zeros((128, W_COLS), np.float16)
    bias_all = np.zeros((128, BIAS_COLS), np.float32)
    for c, ((js, je), o1, o2, o3) in enumerate(zip(CHUNKS, W1_OFF, W2_OFF, W3_OFF)):
        nj = je - js
        r0 = 64 * (c % 2)
        for t, j in enumerate(range(js, je)):
            w_all[r0 + t * 9 : r0 + (t + 1) * 9, o1 + t * 18 : o1 + (t + 1) * 18] = w1[j]
            w_all[t * 18 : (t + 1) * 18, o2 + t * 32 : o2 + (t + 1) * 32] = w2[j]
            w_all[t * 32 : (t + 1) * 32, o3 + t * 8 : o3 + (t + 1) * 8] = w3[j]
        # eye folded into the L1 bias: b1 - (w1[0] + w1[4] + w1[8]) per joint
        b1e = b1[js:je] - w1[js:je][:, [0, 4, 8], :].sum(axis=1)
        bias_all[0 : 18 * nj, c] = b1e.reshape(-1)
        bias_all[0 : 32 * nj, 6 + c] = b2[js:je].reshape(-1)
        if c < 4:
            bias_all[32 * c : 32 * c + 8 * nj, 12] = b3[js:je].reshape(-1)
        else:
            bias_all[32 * (c - 4) : 32 * (c - 4) + 8 * nj, 13] = b3[js:je].reshape(-1)
    # ones selection matrices for S = sum_j coff: rows (j, k) -> col k
    for jk in range(128):
        w_all[jk, EA_OFF + jk % 8] = 1.0
    for jk in range(56):
        w_all[jk, EB_OFF + jk % 8] = 1.0

    return pose_t, basisa_l, bfma_l, bfmb_l, w_all, bias_all, a_lists, b_lists


def _in_maps(pose, basis, mask, w1, b1, w2, b2, w3, b3):
    pose_t, basisa_l, bfma_l, bfmb_l, w_all, bias_all, a_lists, b_lists = _pack_host(
        np.asarray(pose, np.float32),
        np.asarray(basis, np.float32),
        np.asarray(mask, np.float32),
        np.asarray(w1, np.float32),
        np.asarray(b1, np.float32),
        np.asarray(w2, np.float32),
        np.asarray(b2, np.float32),
        np.asarray(w3, np.float32),
        np.asarray(b3, np.float32),
    )
    _CACHED["perm"] = (a_lists, b_lists)
    maps = []
    for i in range(8):
        maps.append(
            {
                "pose_t": pose_t,
                "basisa_t": basisa_l[i],
                "bfmb_a_t": bfma_l[i],
                "bfmb_b_t": bfmb_l[i],
                "w_all": w_all,
                "bias_all": bias_all,
            }
        )
    return maps


def kernel(pose, basis, mask, w1, b1, w2, b2, w3, b3):
    from concourse.bass_utils import run_bass_kernel_spmd

    if "nc" not in _CACHED:
        _CACHED["nc"] = _build_nc()
    nc = _CACHED["nc"]

    maps = _in_maps(pose, basis, mask, w1, b1, w2, b2, w3, b3)
    r = run_bass_kernel_spmd(nc, maps, core_ids=list(range(8)))
    a_lists, b_lists = _CACHED["perm"]
    out = np.empty((BATCH, N_VERT, 3), np.float32)
    for i in range(8):
        ri = np.asarray(r.results[i]["res"]).astype(np.float32)
        na, nb = len(a_lists[i]), len(b_lists[i])
        out[:, a_lists[i]] = ri[:, :A3].reshape(BATCH, NA, 3)[:, :na]
        out[:, b_lists[i]] = ri[:, A3:].reshape(BATCH, NB_SLOTS, 3)[:, :nb]
    return np.ascontiguousarray(out)


# revision 18
# speedup vs baseline: 1.0369x; 1.0369x over previous
"""BlendShapes model kernel for 8 Trainium2 NeuronCores.

Computation (reference):
    pose_repr = pose[:, 1:].reshape(B, 23, 9) - eye      # (B, J, 9)
    per-joint MLP 9 -> 18 -> 32 -> 8 (ReLU between)      # coff (B, J, 8)
    basis_full = basis[:, None] * mask[:, :, None, None]  # (V, J, 8, 3)
    res = einsum('bjk,vjkc->bvc', coff, basis_full)       # (B, V, 3)

Mapping:
  - Vertices are sharded across the 8 cores; each core owns 512 "A" slots
    and 352 "B" slots (864 * 8 = 6912 >= V). A-slots hold vertices whose
    mask row is all ones: for those res = einsum('bk,vkc->bvc', S, basis)
    with S[b,k] = sum_j coff[b,j,k], a K=8 matmul that streams each output
    column ONCE instead of twice (K=184 split 128+56). All other vertices
    (and all-ones overflow) go to B-slots computed with the exact
    mask-folded K=184 path. The host assembles per-core basisA / bfmB
    panels (scaled by 2^13 so f16 stays normal; basis ~1e-4) and scatters
    the result back by vertex index.
  - S is computed on the PE with a ones-selection matrix packed next to the
    MLP weights (K=128 + K=56 accumulated).
  - The eye subtraction is folded into the L1 bias on the host.
  - MLP joints are processed in chunks of 4 (3 for the tail) with
    block-diagonal weights; pose/h1/h2 live in wide tiles with one col
    block per chunk. Odd L1 chunks sit at partition base 64 (pose packed
    [100, 3072]) so the pose DMA spans ~13 of 16 SDMA engines instead
    of 5. L3 outputs land at 32-row offsets of two shared PSUM tiles
    (matmul tile_position), so one bias-add epilogue per half yields
    coffT directly.
  - PSUM-accumulated K chunks evacuate via ACT/DVE with the exact 2^-13
    descale into f16 output (upcast to f32 on host). Input and output DMAs
    are split across the two HWDGE queues (sync + scalar).
"""

import numpy as np

N_VERT, N_JOINT, BPJ, BATCH = 6890, 23, 8, 1024
VC = 864  # vertex slots per core: 512 A + 352 B
VC3 = VC * 3  # 2592
NA, NB_SLOTS = 512, 352
A3, B3 = NA * 3, NB_SLOTS * 3  # 1536, 1056
NB = BATCH // 128  # 8 b-tiles

# Unified joint chunking for the MLP block-diagonal weights.
CHUNKS = [(0, 4), (4, 8), (8, 12), (12, 16), (16, 20), (20, 23)]


def _offsets(mpj):
    offs, col = [], 0
    for js, je in CHUNKS:
        offs.append(col)
        col += (je - js) * mpj
    return offs, col


W1_OFF, W1_TOT = _offsets(18)  # 414
W2_OFF, W2_TOT = _offsets(32)  # 736
W3_OFF, W3_TOT = _offsets(8)   # 184
W2_OFF = [W1_TOT + o for o in W2_OFF]
W3_OFF = [W1_TOT + W2_TOT + o for o in W3_OFF]
EA_OFF = W1_TOT + W2_TOT + W3_TOT  # ones matrix [128, 8] for S (K=128 part)
EB_OFF = EA_OFF + 8                # ones matrix [56, 8] (K=56 part)
W_COLS = EB_OFF + 8  # 1350

# bias_all columns: [0:6] L1 bias (eye folded in), [6:12] L2 bias,
# [12] L3 bias chunks 0-3 stacked at 32-row strides, [13] chunks 4-5.
BIAS_COLS = 14
BSCALE = 8192.0  # 2**13, exact in f16
DESCALE = 1.0 / 8192.0  # exact in f32

_CACHED = {}


def _build_nc():
    import concourse.tile as tile
    from concourse import bacc, mybir
    from contextlib import ExitStack

    dt = mybir.dt
    f32, f16 = dt.float32, dt.float16
    AF = mybir.ActivationFunctionType
    ALU = mybir.AluOpType

    nc = bacc.Bacc(None, target_bir_lowering=False)

    pose_t = nc.dram_tensor("pose_t", [100, 3072], f16, kind="ExternalInput")
    basisa_t = nc.dram_tensor("basisa_t", [BPJ, A3], f16, kind="ExternalInput")
    bfmb_a_t = nc.dram_tensor("bfmb_a_t", [128, B3], f16, kind="ExternalInput")
    bfmb_b_t = nc.dram_tensor("bfmb_b_t", [56, B3], f16, kind="ExternalInput")
    w_all_t = nc.dram_tensor("w_all", [128, W_COLS], f16, kind="ExternalInput")
    bias_t = nc.dram_tensor("bias_all", [128, BIAS_COLS], f32, kind="ExternalInput")
    res = nc.dram_tensor("res", [BATCH, VC3], f16, kind="ExternalOutput")

    with ExitStack() as ctx:
        tc = ctx.enter_context(tile.TileContext(nc))
        const = ctx.enter_context(tc.tile_pool(name="const", bufs=1))
        work = ctx.enter_context(tc.tile_pool(name="work", bufs=1))
        outp = ctx.enter_context(tc.tile_pool(name="outp", bufs=3))
        pmlp = ctx.enter_context(tc.tile_pool(name="pmlp", bufs=4, space="PSUM"))
        pmain = ctx.enter_context(tc.tile_pool(name="pmain", bufs=2, space="PSUM"))

        # ---- input DMAs split across the two HWDGE queues, critical-path
        # tensors first on each. pose rows [0:36] + [64:100] land on
        # disjoint SDMA engine groups; the weights stream layer by layer so
        # L1 can start as soon as its own block arrives.
        pose_sb = const.tile([100, 3072], f16, tag="pose")
        nc.sync.dma_start(out=pose_sb[0:36, :], in_=pose_t[0:36, :])
        nc.sync.dma_start(out=pose_sb[64:100, :], in_=pose_t[64:100, :])
        w_sb = const.tile([128, W_COLS], f16, tag="w")
        nc.scalar.dma_start(out=w_sb[:, 0:W1_TOT], in_=w_all_t[:, 0:W1_TOT])
        bias_sb = const.tile([128, BIAS_COLS], f32, tag="bias")
        nc.scalar.dma_start(out=bias_sb[:], in_=bias_t[:, :])
        nc.scalar.dma_start(
            out=w_sb[:, W1_TOT:W_COLS], in_=w_all_t[:, W1_TOT:W_COLS]
        )
        bfmb_a = const.tile([128, B3], f16, tag="bfmb_a")
        nc.scalar.dma_start(out=bfmb_a[:], in_=bfmb_a_t[:, :])
        bfmb_b = const.tile([56, B3], f16, tag="bfmb_b")
        nc.scalar.dma_start(out=bfmb_b[:], in_=bfmb_b_t[:, :])
        basisa = const.tile([BPJ, A3], f16, tag="basisa")
        nc.scalar.dma_start(out=basisa[:], in_=basisa_t[:, :])

        h1 = work.tile([72, 6 * 1024], f16, tag="h1")
        h2 = work.tile([128, 6 * 1024], f16, tag="h2")
        coffT_a = work.tile([128, BATCH], f16, tag="coffT_a")
        coffT_b = work.tile([56, BATCH], f16, tag="coffT_b")
        st_sb = work.tile([BPJ, BATCH], f16, tag="st")

        def mlp_epilogue(use_act, dst, ps, bias_ap):
            # ReLU(x + b); split between ACT and DVE so the PSUM chain
            # advances two tiles per epilogue latency.
            if use_act:
                nc.scalar.activation(dst, ps, AF.Relu, bias=bias_ap)
            else:
                nc.vector.tensor_scalar(
                    out=dst, in0=ps, scalar1=bias_ap, scalar2=0.0,
                    op0=ALU.add, op1=ALU.max,
                )

        # even chunks first: they only need the first pose DMA slice, so L1
        # starts ~1.3 us before the odd slice lands.
        CORDER = [0, 2, 4, 1, 3, 5]

        def mlp_half(h):
            # L1: 9nj -> 18nj. Odd chunks live at partition base 64 in both
            # pose and w so the pose DMA spans more partitions.
            for ci, c in enumerate(CORDER):
                js, je = CHUNKS[c]
                nj = je - js
                K, M = 9 * nj, 18 * nj
                off = W1_OFF[c]
                r0 = 64 * (c % 2)
                cs = slice(1024 * (c // 2) + 512 * h, 1024 * (c // 2) + 512 * h + 512)
                hcs = slice(1024 * c + 512 * h, 1024 * c + 512 * h + 512)
                ps = pmlp.tile([128, 512], f32, tag="psmlp", name=f"ps1_{c}_{h}")
                nc.tensor.matmul(
                    ps[0:M, :], lhsT=w_sb[r0 : r0 + K, off : off + M],
                    rhs=pose_sb[r0 : r0 + K, cs], start=True, stop=True,
                    tile_position=(r0, 0),
                )
                mlp_epilogue(ci % 2 == 0, h1[0:M, hcs], ps[0:M, :], bias_sb[0:M, c : c + 1])
            # L2: 18nj -> 32nj
            for ci, c in enumerate(CORDER):
                js, je = CHUNKS[c]
                nj = je - js
                K, M = 18 * nj, 32 * nj
                off = W2_OFF[c]
                hcs = slice(1024 * c + 512 * h, 1024 * c + 512 * h + 512)
                ps = pmlp.tile([128, 512], f32, tag="psmlp", name=f"ps2_{c}_{h}")
                nc.tensor.matmul(
                    ps[0:M, :], lhsT=w_sb[0:K, off : off + M], rhs=h1[0:K, hcs],
                    start=True, stop=True,
                )
                mlp_epilogue(ci % 2 == 1, h2[0:M, hcs], ps[0:M, :], bias_sb[0:M, 6 + c : 7 + c])
            # L3: 32nj -> 8nj, written at 32-row offsets of two shared PSUM
            # tiles so the (j, k) rows of coffT form directly; one bias-add
            # epilogue per tile replaces per-chunk epilogues + merge DMAs.
            psA = pmlp.tile([128, 512], f32, tag="psmlp", name=f"ps3a_{h}")
            psB = pmlp.tile([128, 512], f32, tag="psmlp", name=f"ps3b_{h}")
            for c in CORDER:
                js, je = CHUNKS[c]
                nj = je - js
                K, M = 32 * nj, 8 * nj
                off = W3_OFF[c]
                hcs = slice(1024 * c + 512 * h, 1024 * c + 512 * h + 512)
                dstp, r0 = (psA, 32 * c) if c < 4 else (psB, 32 * (c - 4))
                nc.tensor.matmul(
                    dstp[r0 : r0 + M, :], lhsT=w_sb[0:K, off : off + M],
                    rhs=h2[0:K, hcs], start=True, stop=True,
                    tile_position=(0, r0),
                )
            hs = slice(512 * h, 512 * h + 512)
            nc.vector.tensor_scalar(
                out=coffT_a[:, hs], in0=psA[:], scalar1=bias_sb[0:128, 12:13],
                scalar2=None, op0=ALU.add,
            )
            nc.vector.tensor_scalar(
                out=coffT_b[:, hs], in0=psB[0:56, :], scalar1=bias_sb[0:56, 13:14],
                scalar2=None, op0=ALU.add,
            )
            # S[b,k] = sum_j coff[b,j,k] via ones matmuls (for the A path).
            ps_st = pmlp.tile([128, 512], f32, tag="psmlp", name=f"ps_st_{h}")
            nc.tensor.matmul(
                ps_st[0:BPJ, :], lhsT=w_sb[0:128, EA_OFF : EA_OFF + BPJ],
                rhs=coffT_a[:, hs], start=True, stop=False,
            )
            nc.tensor.matmul(
                ps_st[0:BPJ, :], lhsT=w_sb[0:56, EB_OFF : EB_OFF + BPJ],
                rhs=coffT_b[:, hs], start=False, stop=True,
            )
            nc.vector.tensor_scalar(
                out=st_sb[:, hs], in0=ps_st[0:BPJ, :], scalar1=1.0,
                scalar2=None, op0=ALU.mult,
            )

        def main_btile(bt):
            bsl = slice(bt * 128, (bt + 1) * 128)
            ostrip = outp.tile([128, VC3], f16, tag="ostrip", name=f"ostrip_{bt}")

            def evac(k, osl, ps, psl):
                if k % 2 == 0:
                    nc.scalar.activation(ostrip[:, osl], ps[:, psl], AF.Copy, scale=DESCALE)
                else:
                    nc.vector.tensor_scalar(
                        out=ostrip[:, osl], in0=ps[:, psl], scalar1=DESCALE,
                        scalar2=None, op0=ALU.mult,
                    )

            k0 = bt * 3
            # pair 0: A columns 0:1024, K=8 single pass
            ps = pmain.tile([128, 1024], f32, tag="ps", name=f"ps_{bt}_0")
            nc.tensor.matmul(
                ps[:, 0:512], lhsT=st_sb[:, bsl], rhs=basisa[:, 0:512],
                start=True, stop=True,
            )
            nc.tensor.matmul(
                ps[:, 512:1024], lhsT=st_sb[:, bsl], rhs=basisa[:, 512:1024],
                start=True, stop=True,
            )
            evac(k0, slice(0, 1024), ps, slice(0, 1024))
            # pair 1: A columns 1024:1536 (bank 0) + B columns 0:512 (bank 1)
            ps = pmain.tile([128, 1024], f32, tag="ps", name=f"ps_{bt}_1")
            nc.tensor.matmul(
                ps[:, 0:512], lhsT=st_sb[:, bsl], rhs=basisa[:, 1024:1536],
                start=True, stop=True,
            )
            nc.tensor.matmul(
                ps[:, 512:1024], lhsT=coffT_a[:, bsl], rhs=bfmb_a[:, 0:512],
                start=True, stop=False,
            )
            nc.tensor.matmul(
                ps[:, 512:1024], lhsT=coffT_b[:, bsl], rhs=bfmb_b[:, 0:512],
                start=False, stop=True,
            )
            evac(k0 + 1, slice(1024, 2048), ps, slice(0, 1024))
            # pair 2: B columns 512:1056 (512 + 32 tail), two K passes with
            # shared weight loads per pass
            ps = pmain.tile([128, 1024], f32, tag="ps", name=f"ps_{bt}_2")
            nc.tensor.matmul(
                ps[:, 0:512], lhsT=coffT_a[:, bsl], rhs=bfmb_a[:, 512:1024],
                start=True, stop=False,
            )
            nc.tensor.matmul(
                ps[:, 512:544], lhsT=coffT_a[:, bsl], rhs=bfmb_a[:, 1024:1056],
                start=True, stop=False,
            )
            nc.tensor.matmul(
                ps[:, 0:512], lhsT=coffT_b[:, bsl], rhs=bfmb_b[:, 512:1024],
                start=False, stop=True,
            )
            nc.tensor.matmul(
                ps[:, 512:544], lhsT=coffT_b[:, bsl], rhs=bfmb_b[:, 1024:1056],
                start=False, stop=True,
            )
            evac(k0 + 2, slice(2048, 2592), ps, slice(0, 544))

            # output DMA: one per b-tile, alternating queues; the last two
            # b-tiles stream per-pair strips to shorten the tail.
            if bt < 6:
                eng = nc.sync if bt % 2 == 0 else nc.scalar
                eng.dma_start(out=res[bsl, :], in_=ostrip[:])
            else:
                for p, (o0, o1) in enumerate([(0, 1024), (1024, 2048), (2048, 2592)]):
                    eng = nc.sync if (bt + p) % 2 == 0 else nc.scalar
                    eng.dma_start(out=res[bsl, o0:o1], in_=ostrip[:, o0:o1])

        mlp_half(0)
        for bt in range(4):
            main_btile(bt)
        mlp_half(1)
        for bt in range(4, NB):
            main_btile(bt)

    nc.finalize()
    return nc


def _pack_host(pose, basis, mask, w1, b1, w2, b2, w3, b3):
    B = BATCH
    pr = pose[:, 1:].reshape(B, N_JOINT, 9)  # (b, j, i)
    pose_t = np.zeros((100, 3072), np.float16)
    for c, (js, je) in enumerate(CHUNKS):
        nj = je - js
        r0 = 64 * (c % 2)
        c0 = 1024 * (c // 2)
        pose_t[r0 : r0 + 9 * nj, c0 : c0 + 1024] = (
            pr[:, js:je, :].reshape(B, 9 * nj).T.astype(np.float16)
        )
    # rows 36:64 unused (partition alignment for the odd-chunk stack)

    # vertex assignment: all-ones-mask vertices -> A slots (K=8 path),
    # everything else (+ overflow) -> B slots (exact mask-folded path).
    ones_v = np.nonzero((mask > 0.5).all(axis=1))[0]
    rest_v = np.nonzero(~(mask > 0.5).all(axis=1))[0]
    a_sel = ones_v[: 8 * NA]
    b_sel = np.concatenate([ones_v[8 * NA :], rest_v])
    if len(b_sel) > 8 * NB_SLOTS:
        raise ValueError(
            f"B-slot overflow: {len(b_sel)} masked vertices > {8 * NB_SLOTS} slots"
        )
    # round-robin-balanced contiguous split across cores
    a_cnt = [len(a_sel) // 8 + (1 if i < len(a_sel) % 8 else 0) for i in range(8)]
    b_cnt = [len(b_sel) // 8 + (1 if i < len(b_sel) % 8 else 0) for i in range(8)]
    assert max(b_cnt) <= NB_SLOTS and max(a_cnt) <= NA
    a_lists, b_lists = [], []
    ai = bi = 0
    for i in range(8):
        a_lists.append(a_sel[ai : ai + a_cnt[i]])
        b_lists.append(b_sel[bi : bi + b_cnt[i]])
        ai += a_cnt[i]
        bi += b_cnt[i]

    basisT3 = basis.transpose(1, 0, 2).reshape(BPJ, N_VERT * 3)  # (8, V*3) rows k
    m3 = np.repeat(mask.T, 3, axis=1)  # (23, V*3)
    bfm_full = (m3[:, None, :] * basisT3[None, :, :] * BSCALE).reshape(
        N_JOINT * BPJ, N_VERT * 3
    )
    basis_s = basisT3 * BSCALE

    basisa_l, bfma_l, bfmb_l = [], [], []
    for i in range(8):
        av, bv = a_lists[i], b_lists[i]
        acols = (av[:, None] * 3 + np.arange(3)[None, :]).reshape(-1)
        bcols = (bv[:, None] * 3 + np.arange(3)[None, :]).reshape(-1)
        ba = np.zeros((BPJ, A3), np.float16)
        ba[:, : len(acols)] = basis_s[:, acols]
        bb = np.zeros((N_JOINT * BPJ, B3), np.float16)
        bb[:, : len(bcols)] = bfm_full[:, bcols]
        basisa_l.append(ba)
        bfma_l.append(np.ascontiguousarray(bb[0:128]))
        bfmb_l.append(np.ascontiguousarray(bb[128:184]))

    w_all = np.zeros((128, W_COLS), np.float16)
    bias_all = np.zeros((128, BIAS_COLS), np.float32)
    for c, ((js, je), o1, o2, o3) in enumerate(zip(CHUNKS, W1_OFF, W2_OFF, W3_OFF)):
        nj = je - js
        r0 = 64 * (c % 2)
        for t, j in enumerate(range(js, je)):
            w_all[r0 + t * 9 : r0 + (t + 1) * 9, o1 + t * 18 : o1 + (t + 1) * 18] = w1[j]
            w_all[t * 18 : (t + 1) * 18, o2 + t * 32 : o2 + (t + 1) * 32] = w2[j]
            w_all[t * 32 : (t + 1) * 32, o3 + t * 8 : o3 + (t + 1) * 8] = w3[j]
        # eye folded into the L1 bias: b1 - (w1[0] + w1[4] + w1[8]) per joint
        b1e = b1[js:je] - w1[js:je][:, [0, 4, 8], :].sum(axis=1)
        bias_all[0 : 18 * nj, c] = b1e.reshape(-1)
        bias_all[0 : 32 * nj, 6 + c] = b2[js:je].reshape(-1)
        if c < 4:
            bias_all[32 * c : 32 * c + 8 * nj, 12] = b3[js:je].reshape(-1)
        else:
            bias_all[32 * (c - 4) : 32 * (c - 4) + 8 * nj, 13] = b3[js:je].reshape(-1)
    # ones selection matrices for S = sum_j coff: rows (j, k) -> col k
    for jk in range(128):
        w_all[jk, EA_OFF + jk % 8] = 1.0
    for jk in range(56):
        w_all[jk, EB_OFF + jk % 8] = 1.0

    return pose_t, basisa_l, bfma_l, bfmb_l, w_all, bias_all, a_lists, b_lists


def _in_maps(pose, basis, mask, w1, b1, w2, b2, w3, b3):
    pose_t, basisa_l, bfma_l, bfmb_l, w_all, bias_all, a_lists, b_lists = _pack_host(
        np.asarray(pose, np.float32),
        np.asarray(basis, np.float32),
        np.asarray(mask, np.float32),
        np.asarray(w1, np.float32),
        np.asarray(b1, np.float32),
        np.asarray(w2, np.float32),
        np.asarray(b2, np.float32),
        np.asarray(w3, np.float32),
        np.asarray(b3, np.float32),
    )
    _CACHED["perm"] = (a_lists, b_lists)
    maps = []
    for i in range(8):
        maps.append(
            {
                "pose_t": pose_t,
                "basisa_t": basisa_l[i],
                "bfmb_a_t": bfma_l[i],
                "bfmb_b_t": bfmb_l[i],
                "w_all": w_all,
                "bias_all": bias_all,
            }
        )
    return maps


def kernel(pose, basis, mask, w1, b1, w2, b2, w3, b3):
    from concourse.bass_utils import run_bass_kernel_spmd

    if "nc" not in _CACHED:
        _CACHED["nc"] = _build_nc()
    nc = _CACHED["nc"]

    maps = _in_maps(pose, basis, mask, w1, b1, w2, b2, w3, b3)
    r = run_bass_kernel_spmd(nc, maps, core_ids=list(range(8)))
    a_lists, b_lists = _CACHED["perm"]
    out = np.empty((BATCH, N_VERT, 3), np.float32)
    for i in range(8):
        ri = np.asarray(r.results[i]["res"]).astype(np.float32)
        na, nb = len(a_lists[i]), len(b_lists[i])
        out[:, a_lists[i]] = ri[:, :A3].reshape(BATCH, NA, 3)[:, :na]
        out[:, b_lists[i]] = ri[:, A3:].reshape(BATCH, NB_SLOTS, 3)[:, :nb]
    return np.ascontiguousarray(out)


# revision 24
# speedup vs baseline: 1.0493x; 1.0119x over previous
"""BlendShapes model kernel for 8 Trainium2 NeuronCores.

Computation (reference):
    pose_repr = pose[:, 1:].reshape(B, 23, 9) - eye      # (B, J, 9)
    per-joint MLP 9 -> 18 -> 32 -> 8 (ReLU between)      # coff (B, J, 8)
    basis_full = basis[:, None] * mask[:, :, None, None]  # (V, J, 8, 3)
    res = einsum('bjk,vjkc->bvc', coff, basis_full)       # (B, V, 3)

Mapping:
  - Vertices are sharded across the 8 cores; each core owns 512 "A" slots
    and 352 "B" slots (864 * 8 = 6912 >= V). A-slots hold vertices whose
    mask row is all ones: for those res = einsum('bk,vkc->bvc', S, basis)
    with S[b,k] = sum_j coff[b,j,k], a K=8 matmul that streams each output
    column ONCE instead of twice (K=184 split 128+56). All other vertices
    (and all-ones overflow) go to B-slots computed with the exact
    mask-folded K=184 path. The host assembles per-core basisA / bfmB
    panels (scaled by 2^13 so f16 stays normal; basis ~1e-4) and scatters
    the result back by vertex index.
  - S is computed on the PE with a ones-selection matrix packed next to the
    MLP weights (K=128 + K=56 accumulated).
  - The eye subtraction is folded into the L1 bias on the host.
  - MLP joints are processed in chunks of 4 (3 for the tail) with
    block-diagonal weights; pose/h1/h2 live in wide tiles with one col
    block per chunk. Odd L1 chunks sit at partition base 64 (pose packed
    [100, 3072]) so the pose DMA spans ~13 of 16 SDMA engines instead
    of 5. L3 outputs land at 32-row offsets of two shared PSUM tiles
    (matmul tile_position), so one bias-add epilogue per half yields
    coffT directly.
  - PSUM-accumulated K chunks evacuate via ACT/DVE with the exact 2^-13
    descale into f16 output (upcast to f32 on host). Input and output DMAs
    are split across the two HWDGE queues (sync + scalar).
"""

import numpy as np

N_VERT, N_JOINT, BPJ, BATCH = 6890, 23, 8, 1024
VC = 864  # vertex slots per core: 512 A + 352 B
VC3 = VC * 3  # 2592
NA, NB_SLOTS = 512, 352
A3, B3 = NA * 3, NB_SLOTS * 3  # 1536, 1056
NB = BATCH // 128  # 8 b-tiles

# Unified joint chunking for the MLP block-diagonal weights.
CHUNKS = [(0, 4), (4, 8), (8, 12), (12, 16), (16, 20), (20, 23)]


def _offsets(mpj):
    offs, col = [], 0
    for js, je in CHUNKS:
        offs.append(col)
        col += (je - js) * mpj
    return offs, col


W1_OFF, W1_TOT = _offsets(18)  # 414
W2_OFF, W2_TOT = _offsets(32)  # 736
W3_OFF, W3_TOT = _offsets(8)   # 184
W2_OFF = [W1_TOT + o for o in W2_OFF]
W3_OFF = [W1_TOT + W2_TOT + o for o in W3_OFF]
EA_OFF = W1_TOT + W2_TOT + W3_TOT  # ones matrix [128, 8] for S (K=128 part)
EB_OFF = EA_OFF + 8                # ones matrix [56, 8] (K=56 part)
W_COLS = EB_OFF + 8  # 1350

# bias_all columns: [0:6] L1 bias (eye folded in), [6:12] L2 bias,
# [12] L3 bias chunks 0-3 stacked at 32-row strides, [13] chunks 4-5.
BIAS_COLS = 14
BSCALE = 8192.0  # 2**13, exact in f16
DESCALE = 1.0 / 8192.0  # exact in f32

_CACHED = {}


def _build_nc():
    import concourse.tile as tile
    from concourse import bacc, mybir
    from contextlib import ExitStack

    dt = mybir.dt
    f32, f16 = dt.float32, dt.float16
    AF = mybir.ActivationFunctionType
    ALU = mybir.AluOpType

    nc = bacc.Bacc(None, target_bir_lowering=False)

    pose_t = nc.dram_tensor("pose_t", [100, 3072], f16, kind="ExternalInput")
    basisa_t = nc.dram_tensor("basisa_t", [BPJ, A3], f16, kind="ExternalInput")
    bfmb_a_t = nc.dram_tensor("bfmb_a_t", [128, B3], f16, kind="ExternalInput")
    bfmb_b_t = nc.dram_tensor("bfmb_b_t", [56, B3], f16, kind="ExternalInput")
    w_all_t = nc.dram_tensor("w_all", [128, W_COLS], f16, kind="ExternalInput")
    bias_t = nc.dram_tensor("bias_all", [128, BIAS_COLS], f32, kind="ExternalInput")
    res = nc.dram_tensor("res", [BATCH, VC3], f16, kind="ExternalOutput")

    with ExitStack() as ctx:
        tc = ctx.enter_context(tile.TileContext(nc))
        const = ctx.enter_context(tc.tile_pool(name="const", bufs=1))
        work = ctx.enter_context(tc.tile_pool(name="work", bufs=1))
        outp = ctx.enter_context(tc.tile_pool(name="outp", bufs=3))
        pmlp = ctx.enter_context(tc.tile_pool(name="pmlp", bufs=4, space="PSUM"))
        pmain = ctx.enter_context(tc.tile_pool(name="pmain", bufs=2, space="PSUM"))

        # ---- input DMAs split across the two HWDGE queues, critical-path
        # tensors first on each. pose rows [0:36] + [64:100] land on
        # disjoint SDMA engine groups; the weights stream layer by layer so
        # L1 can start as soon as its own block arrives.
        # pose: one DMA per (row-stack, chunk-block) so L1 chunk c can start
        # as soon as its own 72 KB block lands; even/odd stacks drain on
        # disjoint SDMA engine groups in parallel.
        pose_sb = const.tile([100, 3072], f16, tag="pose")
        for blk in range(3):
            cs = slice(1024 * blk, 1024 * blk + 1024)
            nc.sync.dma_start(out=pose_sb[0:36, cs], in_=pose_t[0:36, cs])
        for blk in range(3):
            cs = slice(1024 * blk, 1024 * blk + 1024)
            nc.sync.dma_start(out=pose_sb[64:100, cs], in_=pose_t[64:100, cs])
        basisa = const.tile([BPJ, A3], f16, tag="basisa")
        nc.sync.dma_start(out=basisa[:], in_=basisa_t[:, :])
        w_sb = const.tile([128, W_COLS], f16, tag="w")
        nc.scalar.dma_start(out=w_sb[:, 0:W1_TOT], in_=w_all_t[:, 0:W1_TOT])
        bias_sb = const.tile([128, BIAS_COLS], f32, tag="bias")
        nc.scalar.dma_start(out=bias_sb[:], in_=bias_t[:, :])
        nc.scalar.dma_start(
            out=w_sb[:, W1_TOT:W_COLS], in_=w_all_t[:, W1_TOT:W_COLS]
        )
        bfmb_a = const.tile([128, B3], f16, tag="bfmb_a")
        nc.scalar.dma_start(out=bfmb_a[:], in_=bfmb_a_t[:, :])
        bfmb_b = const.tile([56, B3], f16, tag="bfmb_b")
        nc.scalar.dma_start(out=bfmb_b[:], in_=bfmb_b_t[:, :])

        h1 = work.tile([72, 6 * 1024], f16, tag="h1")
        h2 = work.tile([128, 6 * 1024], f16, tag="h2")
        coffT_a = work.tile([128, BATCH], f16, tag="coffT_a")
        coffT_b = work.tile([56, BATCH], f16, tag="coffT_b")
        st_sb = work.tile([BPJ, BATCH], f16, tag="st")

        def mlp_epilogue(use_act, dst, ps, bias_ap):
            # ReLU(x + b); split between ACT and DVE so the PSUM chain
            # advances two tiles per epilogue latency.
            if use_act:
                nc.scalar.activation(dst, ps, AF.Relu, bias=bias_ap)
            else:
                nc.vector.tensor_scalar(
                    out=dst, in0=ps, scalar1=bias_ap, scalar2=0.0,
                    op0=ALU.add, op1=ALU.max,
                )

        def mlp_half(h):
            # L1: 9nj -> 18nj. Odd chunks live at partition base 64 in both
            # pose and w so the pose DMA spans more partitions.
            for c, (js, je) in enumerate(CHUNKS):
                nj = je - js
                K, M = 9 * nj, 18 * nj
                off = W1_OFF[c]
                r0 = 64 * (c % 2)
                cs = slice(1024 * (c // 2) + 512 * h, 1024 * (c // 2) + 512 * h + 512)
                hcs = slice(1024 * c + 512 * h, 1024 * c + 512 * h + 512)
                ps = pmlp.tile([128, 512], f32, tag="psmlp", name=f"ps1_{c}_{h}")
                nc.tensor.matmul(
                    ps[0:M, :], lhsT=w_sb[r0 : r0 + K, off : off + M],
                    rhs=pose_sb[r0 : r0 + K, cs], start=True, stop=True,
                    tile_position=(r0, 0),
                )
                mlp_epilogue(c % 2 == 0, h1[0:M, hcs], ps[0:M, :], bias_sb[0:M, c : c + 1])
            # L2: 18nj -> 32nj
            for c, (js, je) in enumerate(CHUNKS):
                nj = je - js
                K, M = 18 * nj, 32 * nj
                off = W2_OFF[c]
                hcs = slice(1024 * c + 512 * h, 1024 * c + 512 * h + 512)
                ps = pmlp.tile([128, 512], f32, tag="psmlp", name=f"ps2_{c}_{h}")
                nc.tensor.matmul(
                    ps[0:M, :], lhsT=w_sb[0:K, off : off + M], rhs=h1[0:K, hcs],
                    start=True, stop=True,
                )
                mlp_epilogue(c % 2 == 1, h2[0:M, hcs], ps[0:M, :], bias_sb[0:M, 6 + c : 7 + c])
            # L3: 32nj -> 8nj, written at 32-row offsets of two shared PSUM
            # tiles so the (j, k) rows of coffT form directly; one bias-add
            # epilogue per tile replaces per-chunk epilogues + merge DMAs.
            psA = pmlp.tile([128, 512], f32, tag="psmlp", name=f"ps3a_{h}")
            psB = pmlp.tile([128, 512], f32, tag="psmlp", name=f"ps3b_{h}")
            for c, (js, je) in enumerate(CHUNKS):
                nj = je - js
                K, M = 32 * nj, 8 * nj
                off = W3_OFF[c]
                hcs = slice(1024 * c + 512 * h, 1024 * c + 512 * h + 512)
                dstp, r0 = (psA, 32 * c) if c < 4 else (psB, 32 * (c - 4))
                nc.tensor.matmul(
                    dstp[r0 : r0 + M, :], lhsT=w_sb[0:K, off : off + M],
                    rhs=h2[0:K, hcs], start=True, stop=True,
                    tile_position=(0, r0),
                )
            hs = slice(512 * h, 512 * h + 512)
            nc.vector.tensor_scalar(
                out=coffT_a[:, hs], in0=psA[:], scalar1=bias_sb[0:128, 12:13],
                scalar2=None, op0=ALU.add,
            )
            nc.vector.tensor_scalar(
                out=coffT_b[:, hs], in0=psB[0:56, :], scalar1=bias_sb[0:56, 13:14],
                scalar2=None, op0=ALU.add,
            )
            # S[b,k] = sum_j coff[b,j,k] via ones matmuls (for the A path).
            ps_st = pmlp.tile([128, 512], f32, tag="psmlp", name=f"ps_st_{h}")
            nc.tensor.matmul(
                ps_st[0:BPJ, :], lhsT=w_sb[0:128, EA_OFF : EA_OFF + BPJ],
                rhs=coffT_a[:, hs], start=True, stop=False,
            )
            nc.tensor.matmul(
                ps_st[0:BPJ, :], lhsT=w_sb[0:56, EB_OFF : EB_OFF + BPJ],
                rhs=coffT_b[:, hs], start=False, stop=True,
            )
            nc.vector.tensor_scalar(
                out=st_sb[:, hs], in0=ps_st[0:BPJ, :], scalar1=1.0,
                scalar2=None, op0=ALU.mult,
            )

        def main_btile(bt):
            bsl = slice(bt * 128, (bt + 1) * 128)
            ostrip = outp.tile([128, VC3], f16, tag="ostrip", name=f"ostrip_{bt}")

            def evac(k, osl, ps, psl):
                if k % 2 == 0:
                    nc.scalar.activation(ostrip[:, osl], ps[:, psl], AF.Copy, scale=DESCALE)
                else:
                    nc.vector.tensor_scalar(
                        out=ostrip[:, osl], in0=ps[:, psl], scalar1=DESCALE,
                        scalar2=None, op0=ALU.mult,
                    )

            k0 = bt * 3
            # pair 0: A columns 0:1024, K=8 single pass
            ps = pmain.tile([128, 1024], f32, tag="ps", name=f"ps_{bt}_0")
            nc.tensor.matmul(
                ps[:, 0:512], lhsT=st_sb[:, bsl], rhs=basisa[:, 0:512],
                start=True, stop=True,
            )
            nc.tensor.matmul(
                ps[:, 512:1024], lhsT=st_sb[:, bsl], rhs=basisa[:, 512:1024],
                start=True, stop=True,
            )
            evac(k0, slice(0, 1024), ps, slice(0, 1024))
            # pair 1: A columns 1024:1536 (bank 0) + B columns 0:512 (bank 1)
            ps = pmain.tile([128, 1024], f32, tag="ps", name=f"ps_{bt}_1")
            nc.tensor.matmul(
                ps[:, 0:512], lhsT=st_sb[:, bsl], rhs=basisa[:, 1024:1536],
                start=True, stop=True,
            )
            nc.tensor.matmul(
                ps[:, 512:1024], lhsT=coffT_a[:, bsl], rhs=bfmb_a[:, 0:512],
                start=True, stop=False,
            )
            nc.tensor.matmul(
                ps[:, 512:1024], lhsT=coffT_b[:, bsl], rhs=bfmb_b[:, 0:512],
                start=False, stop=True,
            )
            evac(k0 + 1, slice(1024, 2048), ps, slice(0, 1024))
            # pair 2: B columns 512:1056 (512 + 32 tail), two K passes with
            # shared weight loads per pass
            ps = pmain.tile([128, 1024], f32, tag="ps", name=f"ps_{bt}_2")
            nc.tensor.matmul(
                ps[:, 0:512], lhsT=coffT_a[:, bsl], rhs=bfmb_a[:, 512:1024],
                start=True, stop=False,
            )
            nc.tensor.matmul(
                ps[:, 512:544], lhsT=coffT_a[:, bsl], rhs=bfmb_a[:, 1024:1056],
                start=True, stop=False,
            )
            nc.tensor.matmul(
                ps[:, 0:512], lhsT=coffT_b[:, bsl], rhs=bfmb_b[:, 512:1024],
                start=False, stop=True,
            )
            nc.tensor.matmul(
                ps[:, 512:544], lhsT=coffT_b[:, bsl], rhs=bfmb_b[:, 1024:1056],
                start=False, stop=True,
            )
            evac(k0 + 2, slice(2048, 2592), ps, slice(0, 544))

            # output DMA: one per b-tile, alternating queues; the last two
            # b-tiles stream per-pair strips to shorten the tail.
            if bt < 6:
                eng = nc.sync if bt % 2 == 0 else nc.scalar
                eng.dma_start(out=res[bsl, :], in_=ostrip[:])
            else:
                for p, (o0, o1) in enumerate([(0, 1024), (1024, 2048), (2048, 2592)]):
                    eng = nc.sync if (bt + p) % 2 == 0 else nc.scalar
                    eng.dma_start(out=res[bsl, o0:o1], in_=ostrip[:, o0:o1])

        mlp_half(0)
        for bt in range(4):
            main_btile(bt)
        mlp_half(1)
        for bt in range(4, NB):
            main_btile(bt)

    nc.finalize()
    return nc


def _pack_host(pose, basis, mask, w1, b1, w2, b2, w3, b3):
    B = BATCH
    pr = pose[:, 1:].reshape(B, N_JOINT, 9)  # (b, j, i)
    pose_t = np.zeros((100, 3072), np.float16)
    for c, (js, je) in enumerate(CHUNKS):
        nj = je - js
        r0 = 64 * (c % 2)
        c0 = 1024 * (c // 2)
        pose_t[r0 : r0 + 9 * nj, c0 : c0 + 1024] = (
            pr[:, js:je, :].reshape(B, 9 * nj).T.astype(np.float16)
        )
    # rows 36:64 unused (partition alignment for the odd-chunk stack)

    # vertex assignment: all-ones-mask vertices -> A slots (K=8 path),
    # everything else (+ overflow) -> B slots (exact mask-folded path).
    ones_v = np.nonzero((mask > 0.5).all(axis=1))[0]
    rest_v = np.nonzero(~(mask > 0.5).all(axis=1))[0]
    a_sel = ones_v[: 8 * NA]
    b_sel = np.concatenate([ones_v[8 * NA :], rest_v])
    if len(b_sel) > 8 * NB_SLOTS:
        raise ValueError(
            f"B-slot overflow: {len(b_sel)} masked vertices > {8 * NB_SLOTS} slots"
        )
    # round-robin-balanced contiguous split across cores
    a_cnt = [len(a_sel) // 8 + (1 if i < len(a_sel) % 8 else 0) for i in range(8)]
    b_cnt = [len(b_sel) // 8 + (1 if i < len(b_sel) % 8 else 0) for i in range(8)]
    assert max(b_cnt) <= NB_SLOTS and max(a_cnt) <= NA
    a_lists, b_lists = [], []
    ai = bi = 0
    for i in range(8):
        a_lists.append(a_sel[ai : ai + a_cnt[i]])
        b_lists.append(b_sel[bi : bi + b_cnt[i]])
        ai += a_cnt[i]
        bi += b_cnt[i]

    basisT3 = basis.transpose(1, 0, 2).reshape(BPJ, N_VERT * 3)  # (8, V*3) rows k
    m3 = np.repeat(mask.T, 3, axis=1)  # (23, V*3)
    bfm_full = (m3[:, None, :] * basisT3[None, :, :] * BSCALE).reshape(
        N_JOINT * BPJ, N_VERT * 3
    )
    basis_s = basisT3 * BSCALE

    basisa_l, bfma_l, bfmb_l = [], [], []
    for i in range(8):
        av, bv = a_lists[i], b_lists[i]
        acols = (av[:, None] * 3 + np.arange(3)[None, :]).reshape(-1)
        bcols = (bv[:, None] * 3 + np.arange(3)[None, :]).reshape(-1)
        ba = np.zeros((BPJ, A3), np.float16)
        ba[:, : len(acols)] = basis_s[:, acols]
        bb = np.zeros((N_JOINT * BPJ, B3), np.float16)
        bb[:, : len(bcols)] = bfm_full[:, bcols]
        basisa_l.append(ba)
        bfma_l.append(np.ascontiguousarray(bb[0:128]))
        bfmb_l.append(np.ascontiguousarray(bb[128:184]))

    w_all = np.zeros((128, W_COLS), np.float16)
    bias_all = np.zeros((128, BIAS_COLS), np.float32)
    for c, ((js, je), o1, o2, o3) in enumerate(zip(CHUNKS, W1_OFF, W2_OFF, W3_OFF)):
        nj = je - js
        r0 = 64 * (c % 2)
        for t, j in enumerate(range(js, je)):
            w_all[r0 + t * 9 : r0 + (t + 1) * 9, o1 + t * 18 : o1 + (t + 1) * 18] = w1[j]
            w_all[t * 18 : (t + 1) * 18, o2 + t * 32 : o2 + (t + 1) * 32] = w2[j]
            w_all[t * 32 : (t + 1) * 32, o3 + t * 8 : o3 + (t + 1) * 8] = w3[j]
        # eye folded into the L1 bias: b1 - (w1[0] + w1[4] + w1[8]) per joint
        b1e = b1[js:je] - w1[js:je][:, [0, 4, 8], :].sum(axis=1)
        bias_all[0 : 18 * nj, c] = b1e.reshape(-1)
        bias_all[0 : 32 * nj, 6 + c] = b2[js:je].reshape(-1)
        if c < 4:
            bias_all[32 * c : 32 * c + 8 * nj, 12] = b3[js:je].reshape(-1)
        else:
            bias_all[32 * (c - 4) : 32 * (c - 4) + 8 * nj, 13] = b3[js:je].reshape(-1)
    # ones selection matrices for S = sum_j coff: rows (j, k) -> col k
    for jk in range(128):
        w_all[jk, EA_OFF + jk % 8] = 1.0
    for jk in range(56):
        w_all[jk, EB_OFF + jk % 8] = 1.0

    return pose_t, basisa_l, bfma_l, bfmb_l, w_all, bias_all, a_lists, b_lists


def _in_maps(pose, basis, mask, w1, b1, w2, b2, w3, b3):
    pose_t, basisa_l, bfma_l, bfmb_l, w_all, bias_all, a_lists, b_lists = _pack_host(
        np.asarray(pose, np.float32),
        np.asarray(basis, np.float32),
        np.asarray(mask, np.float32),
        np.asarray(w1, np.float32),
        np.asarray(b1, np.float32),
        np.asarray(w2, np.float32),
        np.asarray(b2, np.float32),
        np.asarray(w3, np.float32),
        np.asarray(b3, np.float32),
    )
    _CACHED["perm"] = (a_lists, b_lists)
    maps = []
    for i in range(8):
        maps.append(
            {
                "pose_t": pose_t,
                "basisa_t": basisa_l[i],
                "bfmb_a_t": bfma_l[i],
                "bfmb_b_t": bfmb_l[i],
                "w_all": w_all,
                "bias_all": bias_all,
            }
        )
    return maps


def kernel(pose, basis, mask, w1, b1, w2, b2, w3, b3):
    from concourse.bass_utils import run_bass_kernel_spmd

    if "nc" not in _CACHED:
        _CACHED["nc"] = _build_nc()
    nc = _CACHED["nc"]

    maps = _in_maps(pose, basis, mask, w1, b1, w2, b2, w3, b3)
    r = run_bass_kernel_spmd(nc, maps, core_ids=list(range(8)))
    a_lists, b_lists = _CACHED["perm"]
    out = np.empty((BATCH, N_VERT, 3), np.float32)
    for i in range(8):
        ri = np.asarray(r.results[i]["res"]).astype(np.float32)
        na, nb = len(a_lists[i]), len(b_lists[i])
        out[:, a_lists[i]] = ri[:, :A3].reshape(BATCH, NA, 3)[:, :na]
        out[:, b_lists[i]] = ri[:, A3:].reshape(BATCH, NB_SLOTS, 3)[:, :nb]
    return np.ascontiguousarray(out)
